# revision 9
# baseline (speedup 1.0000x reference)
"""GuidedFilter Trainium2 kernel v2: batch-parallel over 8 NeuronCores.

Per core: img [512,512] bf16, feat [16,512,512] bf16 -> out [16,512,512] bf16
(host casts f32<->bf16). Each 2-D box blur (radius 5, reflect) is two PE
passes against per-128-chunk diagonal blocks of the box matrix B plus tiny
5-wide boundary-correction matmuls, cutting PE streaming 2.4x vs a banded
block decomposition:
  pass1: T1[w,i] = sum_r X[r,w] B[i,r]   (lhsT = X chunks, rhs = B^T blocks)
  pass2: out[i,w'] = sum_w T1[w,i] B[w',w]
Orientation is preserved (out has the input layout). Evacs are fused with the
per-channel elementwise math and spread across Act/DVE/Pool; data DMAs issue
from the idle SP (sync) HWDGE path. PSUM rotates two [128,2048] f32 tiles
(4 banks each); emission software-pipelines phase1(d) with phase2(d-1).
"""
import sys

sys.path.insert(0, "/opt/trn_rl_repo")

import numpy as np
import ml_dtypes

RADIUS = 5
H = W = 512
D = 16
NCORES = 8
U = 1.0 / 121.0
VAR_FLOOR = 1e-6

_BT_OFF = [0, 128, 128, 256]  # rhs col offset of B^T diag block per chunk
_TRI_NEXT = 384               # [5,5] correction blocks
_TRI_PREV = 392


def _box_matrix():
    B = np.zeros((512, 512), np.float32)
    for i in range(512):
        for d in range(-RADIUS, RADIUS + 1):
            j = i + d
            if j < 0:
                j = -j
            elif j > 511:
                j = 1022 - j
            B[i, j] += 1.0
    return B


def _g_packed():
    B = _box_matrix()
    G = np.zeros((128, 512), np.float32)
    G[:, 0:128] = B[0:128, 0:128].T
    G[:, 128:256] = B[128:256, 128:256].T
    G[:, 256:384] = B[384:512, 384:512].T
    # tri_next[s,o] = B[123+o, 128+s]; tri_prev[s,o] = B[128+o, 123+s].
    # tri_prev sits at partitions 123:128 so its matmuls can use a legal
    # base-64 partition slice (PE requires operand base in {0,32,64}).
    G[0:5, 384:389] = B[123:128, 128:133].T
    G[123:128, 392:397] = B[128:133, 123:128].T
    return np.ascontiguousarray(G).astype(ml_dtypes.bfloat16)


def _build_bass():
    import concourse.bass as bass
    import concourse.bacc as bacc
    import concourse.tile as tile
    from concourse import mybir

    f32 = mybir.dt.float32
    bf16 = mybir.dt.bfloat16
    Alu = mybir.AluOpType
    Act = mybir.ActivationFunctionType

    nc = bacc.Bacc("TRN2", target_bir_lowering=False, debug=False,
                   num_devices=NCORES)

    feat_d = nc.dram_tensor("feat", [D, H, W], bf16, kind="ExternalInput").ap()
    img_d = nc.dram_tensor("img", [H, W], bf16, kind="ExternalInput").ap()
    g_d = nc.dram_tensor("gmat", [128, 512], bf16, kind="ExternalInput").ap()
    out_d = nc.dram_tensor("out", [D, H, W], bf16, kind="ExternalOutput").ap()

    def ld(dst, src2d):
        nc.sync.dma_start(
            out=dst.rearrange("p (j w) -> p j w", j=4),
            in_=src2d.rearrange("(j p) w -> p j w", p=128))

    def st(dst2d, src):
        nc.sync.dma_start(
            out=dst2d.rearrange("(j p) w -> p j w", p=128),
            in_=src.rearrange("p (j w) -> p j w", j=4))

    with tile.TileContext(nc) as tc:
        with (
            tc.tile_pool(name="consts", bufs=1) as consts,
            tc.tile_pool(name="shared", bufs=1) as shared,
            tc.tile_pool(name="xin", bufs=3) as xin,
            tc.tile_pool(name="chan", bufs=2) as chan,
            tc.tile_pool(name="t1p", bufs=4) as t1p,
            tc.tile_pool(name="psum", bufs=2, space="PSUM") as psum,
        ):
            G = consts.tile([128, 512], bf16)
            nc.sync.dma_start(out=G[:], in_=g_d)
            I = consts.tile([128, 2048], bf16)
            ld(I, img_d)

            def pass1(Xt, P1):
                """P1[w-chunk c][:, i] = blur-rows of Xt (both [128,2048])."""
                for c in range(4):
                    base = 512 * c
                    for j in range(4):
                        nc.tensor.matmul(
                            P1[:, base + 128 * j: base + 128 * (j + 1)],
                            Xt[:, 512 * j + 128 * c: 512 * j + 128 * c + 128],
                            G[:, _BT_OFF[j]: _BT_OFF[j] + 128],
                            start=(j == 0), stop=False, skip_group_check=True)
                    for j in range(3):
                        nc.tensor.matmul(
                            P1[:, base + 128 * j + 123: base + 128 * (j + 1)],
                            Xt[0:5, 512 * (j + 1) + 128 * c: 512 * (j + 1) + 128 * c + 128],
                            G[0:5, _TRI_NEXT:_TRI_NEXT + 5],
                            start=False, stop=False, skip_group_check=True)
                        nc.tensor.matmul(
                            P1[:, base + 128 * (j + 1): base + 128 * (j + 1) + 5],
                            Xt[64:128, 512 * j + 128 * c: 512 * j + 128 * c + 128],
                            G[64:128, _TRI_PREV:_TRI_PREV + 5],
                            start=False, stop=(j == 2), skip_group_check=True)

            def pass2(T1, P2):
                """P2[i-chunk k][:, w'] = blur-cols of T1."""
                for k in range(4):
                    base = 512 * k
                    for c in range(4):
                        nc.tensor.matmul(
                            P2[:, base + 128 * c: base + 128 * (c + 1)],
                            T1[:, 512 * c + 128 * k: 512 * c + 128 * k + 128],
                            G[:, _BT_OFF[c]: _BT_OFF[c] + 128],
                            start=(c == 0), stop=False, skip_group_check=True)
                    for c in range(3):
                        nc.tensor.matmul(
                            P2[:, base + 128 * c + 123: base + 128 * (c + 1)],
                            T1[0:5, 512 * (c + 1) + 128 * k: 512 * (c + 1) + 128 * k + 128],
                            G[0:5, _TRI_NEXT:_TRI_NEXT + 5],
                            start=False, stop=False, skip_group_check=True)
                        nc.tensor.matmul(
                            P2[:, base + 128 * (c + 1): base + 128 * (c + 1) + 5],
                            T1[64:128, 512 * c + 128 * k: 512 * c + 128 * k + 128],
                            G[64:128, _TRI_PREV:_TRI_PREV + 5],
                            start=False, stop=(c == 2), skip_group_check=True)

            # ---------------- img stage ----------------
            xtiles = {}
            for dd in range(min(2, D)):
                Xt = xin.tile([128, 2048], bf16, tag="x", name=f"x{dd}")
                ld(Xt, feat_d[dd])
                xtiles[dd] = Xt

            I2 = shared.tile([128, 2048], bf16)
            nc.vector.tensor_mul(I2[:], I[:], I[:])

            P1i = psum.tile([128, 2048], f32, tag="ps", name="p1i")
            pass1(I, P1i)
            T1i = t1p.tile([128, 2048], bf16, tag="t1", name="t1i")
            nc.scalar.copy(T1i[:], P1i[:])
            P2i = psum.tile([128, 2048], f32, tag="ps", name="p2i")
            pass2(T1i, P2i)
            mIs = shared.tile([128, 2048], bf16)
            nc.scalar.activation(mIs[:], P2i[:], Act.Copy, 0.0, U)

            P1j = psum.tile([128, 2048], f32, tag="ps", name="p1j")
            pass1(I2, P1j)
            T1j = t1p.tile([128, 2048], bf16, tag="t1", name="t1j")
            nc.scalar.copy(T1j[:], P1j[:])
            P2j = psum.tile([128, 2048], f32, tag="ps", name="p2j")
            pass2(T1j, P2j)

            m2 = shared.tile([128, 2048], f32)
            nc.vector.tensor_mul(m2[:], mIs[:], mIs[:])
            varp = shared.tile([128, 2048], f32)
            nc.vector.scalar_tensor_tensor(
                varp[:], P2j[:], U, m2[:], op0=Alu.mult, op1=Alu.subtract)
            nc.vector.tensor_scalar_max(varp[:], varp[:], VAR_FLOOR)
            R = shared.tile([128, 2048], f32)
            nc.vector.reciprocal_approx_fast(R[:], varp[:])
            RS = shared.tile([128, 2048], bf16)
            nc.vector.tensor_scalar_mul(RS[:], R[:], U)
            mIR = shared.tile([128, 2048], bf16)
            nc.vector.tensor_mul(mIR[:], mIs[:], R[:])

            pd_t = {}
            Pd0 = chan.tile([128, 2048], bf16, tag="pd", name="pd0")
            nc.gpsimd.tensor_mul(Pd0[:], xtiles[0][:], I[:])
            pd_t[0] = Pd0

            mp_t, t2_t, v_t = {}, {}, {}

            def phase1(d):
                if d + 1 < D:
                    Xn = xin.tile([128, 2048], bf16, tag="x", name=f"x{d+1}")
                    ld(Xn, feat_d[d + 1])
                    xtiles[d + 1] = Xn
                X = xtiles[d]
                Pd = pd_t[d]
                P1x = psum.tile([128, 2048], f32, tag="ps", name=f"p1x{d}")
                pass1(X, P1x)
                P1p = psum.tile([128, 2048], f32, tag="ps", name=f"p1p{d}")
                pass1(Pd, P1p)
                T1x = t1p.tile([128, 2048], bf16, tag="t1", name=f"t1x{d}")
                nc.scalar.copy(T1x[:], P1x[:])              # H_X  (Act)
                T1q = t1p.tile([128, 2048], bf16, tag="t1", name=f"t1q{d}")
                nc.scalar.copy(T1q[:], P1p[:])              # H_P  (Act)
                P2x = psum.tile([128, 2048], f32, tag="ps", name=f"p2x{d}")
                pass2(T1x, P2x)
                P2p = psum.tile([128, 2048], f32, tag="ps", name=f"p2p{d}")
                pass2(T1q, P2p)
                mp = chan.tile([128, 2048], bf16, tag="mp", name=f"mp{d}")
                nc.scalar.activation(mp[:], P2x[:], Act.Copy, 0.0, U)  # E_mp (Act)
                mp_t[d] = mp
                return P2p

            ab_t, p2ab_t = {}, {}

            def chain_b(d):
                # b(d): u2(d) ran on Pool late last slot; lands early this slot
                mp = mp_t[d]
                a, u2 = ab_t[d]
                b = chan.tile([128, 2048], bf16, tag="b", name=f"b{d}")
                nc.vector.tensor_sub(b[:], mp[:], u2[:])
                ab_t[d] = (a, b)

            def chain_head(d, P2p):
                mp = mp_t[d]
                t1m = chan.tile([128, 2048], bf16, tag="t1m", name=f"t1m{d}")
                nc.vector.tensor_mul(t1m[:], mp[:], mIR[:])
                t2 = chan.tile([128, 2048], bf16, tag="t2", name=f"t2{d}")
                nc.vector.tensor_mul(t2[:], P2p[:], RS[:])  # E_t2 (DVE)
                a = chan.tile([128, 2048], bf16, tag="a", name=f"a{d}")
                nc.vector.tensor_sub(a[:], t2[:], t1m[:])
                u2 = chan.tile([128, 2048], bf16, tag="u2", name=f"u2{d}")
                nc.gpsimd.tensor_mul(u2[:], a[:], mIs[:])   # Pool
                ab_t[d] = (a, u2)

            def phase2_blur(d):
                a, b = ab_t[d]
                P1a = psum.tile([128, 2048], f32, tag="ps", name=f"p1a{d}")
                pass1(a, P1a)
                P1b = psum.tile([128, 2048], f32, tag="ps", name=f"p1b{d}")
                pass1(b, P1b)
                T1a = t1p.tile([128, 2048], bf16, tag="t1", name=f"t1a{d}")
                nc.scalar.copy(T1a[:], P1a[:])              # H_a  (Act)
                T1b = t1p.tile([128, 2048], bf16, tag="t1", name=f"t1b{d}")
                nc.scalar.copy(T1b[:], P1b[:])              # H_b  (Act)
                P2a = psum.tile([128, 2048], f32, tag="ps", name=f"p2a{d}")
                pass2(T1a, P2a)
                P2b = psum.tile([128, 2048], f32, tag="ps", name=f"p2b{d}")
                pass2(T1b, P2b)
                p2ab_t[d] = (P2a, P2b)

            def evac_vo(d):
                P2a, P2b = p2ab_t[d]
                v = chan.tile([128, 2048], bf16, tag="v", name=f"v{d}")
                nc.vector.scalar_tensor_tensor(
                    v[:], P2a[:], U, I[:], op0=Alu.mult, op1=Alu.mult)  # E_v (DVE)
                o = chan.tile([128, 2048], bf16, tag="o", name=f"o{d}")
                nc.vector.scalar_tensor_tensor(
                    o[:], P2b[:], U, v[:], op0=Alu.mult, op1=Alu.add)   # E_o (DVE)
                st(out_d[d], o)

            def prefetch_pd(d):
                if d < D:
                    Pd = chan.tile([128, 2048], bf16, tag="pd", name=f"pd{d}")
                    nc.gpsimd.tensor_mul(Pd[:], xtiles[d][:], I[:])  # Pool
                    pd_t[d] = Pd

            # Steady-state slot d: phase1(d) | E_v/E_o(d-3) | b(d-1) |
            # chain head (t1m, t2, a)(d) + u2(d) on Pool | blurs of (d-2).
            for d in range(D):
                P2p = phase1(d)
                if d >= 3:
                    evac_vo(d - 3)
                if d >= 1:
                    chain_b(d - 1)
                chain_head(d, P2p)
                if d >= 2:
                    phase2_blur(d - 2)
                prefetch_pd(d + 1)
            chain_b(D - 1)
            phase2_blur(D - 2)
            evac_vo(D - 3)
            phase2_blur(D - 1)
            evac_vo(D - 2)
            evac_vo(D - 1)

    nc.compile()
    return nc


_NC_CACHE = None


def kernel(feat: np.ndarray, img: np.ndarray) -> np.ndarray:
    global _NC_CACHE
    from concourse.bass_utils import run_bass_kernel_spmd

    if _NC_CACHE is None:
        _NC_CACHE = _build_bass()
    nc = _NC_CACHE
    g = _g_packed()
    bf = ml_dtypes.bfloat16
    featb = np.ascontiguousarray(np.asarray(feat, np.float32)).astype(bf)
    imgb = np.ascontiguousarray(np.asarray(img, np.float32)).astype(bf)
    in_maps = [
        {"feat": featb[c], "img": imgb[c, 0], "gmat": g} for c in range(NCORES)
    ]
    res = run_bass_kernel_spmd(nc, in_maps, list(range(NCORES)))
    return np.stack(
        [res.results[c]["out"].astype(np.float32) for c in range(NCORES)], axis=0)


# revision 10
# speedup vs baseline: 1.1740x; 1.1740x over previous
"""GuidedFilter Trainium2 kernel v3: batch-parallel over 8 NeuronCores.

Per core: img [512,512] bf16, feat [16,512,512] bf16 -> out [16,512,512] bf16
(host casts f32<->bf16). Each 2-D box blur (radius 5, reflect) is two PE
passes against per-128-chunk diagonal blocks of the box matrix B plus 5-wide
boundary-correction matmuls (2.4x less PE streaming than a banded block
decomposition):
  pass1: T1[w,i] = sum_r X[r,w] B[i,r]   (lhsT = X chunks, rhs = B^T blocks)
  pass2: out[i,w'] = sum_w T1[w,i] B[w',w]
Passes are emitted as i-halves over [128,1024] PSUM tiles (2 banks) with two
double-buffered tags, so four tiles are in flight and the A->H->C->E evac
ring never serializes the engines. T1 uses an [i-half][w-chunk][256] free
layout so every copy and lhsT slice is contiguous. Evacs are fused into the
per-channel elementwise math on Act/DVE; the two big sbuf multiplies run on
Pool; data DMAs issue from the idle SP (sync) HWDGE path.
"""
import sys

sys.path.insert(0, "/opt/trn_rl_repo")

import numpy as np
import ml_dtypes

RADIUS = 5
H = W = 512
D = 16
NCORES = 8
U = 1.0 / 121.0
VAR_FLOOR = 1e-6

_BT_OFF = [0, 128, 128, 256]  # rhs col offset of B^T diag block per chunk
_TRI_NEXT = 384               # [5,5] corrections; tri_prev at partitions 123:128
_TRI_PREV = 392


def _box_matrix():
    B = np.zeros((512, 512), np.float32)
    for i in range(512):
        for d in range(-RADIUS, RADIUS + 1):
            j = i + d
            if j < 0:
                j = -j
            elif j > 511:
                j = 1022 - j
            B[i, j] += 1.0
    return B


def _g_packed():
    B = _box_matrix()
    G = np.zeros((128, 512), np.float32)
    G[:, 0:128] = B[0:128, 0:128].T
    G[:, 128:256] = B[128:256, 128:256].T
    G[:, 256:384] = B[384:512, 384:512].T
    G[0:5, 384:389] = B[123:128, 128:133].T
    G[123:128, 392:397] = B[128:133, 123:128].T
    return np.ascontiguousarray(G).astype(ml_dtypes.bfloat16)


def _build_bass():
    import concourse.bass as bass
    import concourse.bacc as bacc
    import concourse.tile as tile
    from concourse import mybir

    f32 = mybir.dt.float32
    bf16 = mybir.dt.bfloat16
    Alu = mybir.AluOpType
    Act = mybir.ActivationFunctionType

    nc = bacc.Bacc("TRN2", target_bir_lowering=False, debug=False,
                   num_devices=NCORES)

    feat_d = nc.dram_tensor("feat", [D, H, W], bf16, kind="ExternalInput").ap()
    img_d = nc.dram_tensor("img", [H, W], bf16, kind="ExternalInput").ap()
    g_d = nc.dram_tensor("gmat", [128, 512], bf16, kind="ExternalInput").ap()
    out_d = nc.dram_tensor("out", [D, H, W], bf16, kind="ExternalOutput").ap()

    def ld(dst, src2d):
        nc.sync.dma_start(
            out=dst.rearrange("p (j w) -> p j w", j=4),
            in_=src2d.rearrange("(j p) w -> p j w", p=128))

    def st(dst2d, src):
        nc.sync.dma_start(
            out=dst2d.rearrange("(j p) w -> p j w", p=128),
            in_=src.rearrange("p (j w) -> p j w", j=4))

    with tile.TileContext(nc) as tc:
        with (
            tc.tile_pool(name="consts", bufs=1) as consts,
            tc.tile_pool(name="shared", bufs=1) as shared,
            tc.tile_pool(name="xin", bufs=3) as xin,
            tc.tile_pool(name="chan", bufs=2) as chan,
            tc.tile_pool(name="t1p", bufs=4) as t1p,
            tc.tile_pool(name="psum", bufs=2, space="PSUM") as psum,
        ):
            G = consts.tile([128, 512], bf16)
            nc.sync.dma_start(out=G[:], in_=g_d)
            I = consts.tile([128, 2048], bf16)
            ld(I, img_d)

            def mm(out, lhsT, rhs, start, stop):
                nc.tensor.matmul(out, lhsT, rhs, start=start, stop=stop,
                                 skip_group_check=True)

            def pass1_half(Xt, P1, h):
                """P1 [128,1024] = rows 256h..256h+256 of (B X)^T, [c,i'] layout."""
                for c in range(4):
                    base = 256 * c
                    x0 = 128 * c
                    for jj in (0, 1):
                        j = 2 * h + jj
                        mm(P1[:, base + 128 * jj: base + 128 * (jj + 1)],
                           Xt[:, 512 * j + x0: 512 * j + x0 + 128],
                           G[:, _BT_OFF[j]: _BT_OFF[j] + 128],
                           start=(c % 2 == 0 and jj == 0), stop=False)
                    strips = [(123, 2 * h + 1, 'n'), (128, 2 * h, 'p'),
                              (251, 2, 'n') if h == 0 else (0, 1, 'p')]
                    for si, (off, sc, t) in enumerate(strips):
                        last = (c % 2 == 1 and si == 2)
                        if t == 'n':
                            mm(P1[:, base + off: base + off + 5],
                               Xt[0:5, 512 * sc + x0: 512 * sc + x0 + 128],
                               G[0:5, _TRI_NEXT:_TRI_NEXT + 5],
                               start=False, stop=last)
                        else:
                            mm(P1[:, base + off: base + off + 5],
                               Xt[64:128, 512 * sc + x0: 512 * sc + x0 + 128],
                               G[64:128, _TRI_PREV:_TRI_PREV + 5],
                               start=False, stop=last)

            def pass2_half(T1, P2, g):
                """P2 [128,1024] = out rows (i-chunks 2g,2g+1), [kk,w'] layout."""
                for kk in (0, 1):
                    k = 2 * g + kk
                    base = 512 * kk
                    t0 = 1024 * g + 128 * kk

                    def tsl(c):
                        return slice(t0 + 256 * c, t0 + 256 * c + 128)
                    for c in range(4):
                        mm(P2[:, base + 128 * c: base + 128 * (c + 1)],
                           T1[:, tsl(c)], G[:, _BT_OFF[c]: _BT_OFF[c] + 128],
                           start=(c == 0), stop=False)
                    for c in range(3):
                        mm(P2[:, base + 128 * c + 123: base + 128 * (c + 1)],
                           T1[0:5, tsl(c + 1)],
                           G[0:5, _TRI_NEXT:_TRI_NEXT + 5],
                           start=False, stop=False)
                        mm(P2[:, base + 128 * (c + 1): base + 128 * (c + 1) + 5],
                           T1[64:128, tsl(c)],
                           G[64:128, _TRI_PREV:_TRI_PREV + 5],
                           start=False, stop=(c == 2))

            def blur_p1(Xt, nm):
                """pass1 both halves -> T1 sbuf tile (H on Act)."""
                T1 = t1p.tile([128, 2048], bf16, tag="t1", name=f"t1{nm}")
                ps = []
                for h in (0, 1):
                    P1 = psum.tile([128, 1024], f32, tag="q1", name=f"p1{nm}{h}")
                    pass1_half(Xt, P1, h)
                    ps.append(P1)
                for h in (0, 1):
                    nc.scalar.copy(T1[:, 1024 * h:1024 * (h + 1)], ps[h][:])
                return T1

            def blur_p2(T1, nm):
                """pass2 both halves -> two psum tiles [128,1024]."""
                ps = []
                for g in (0, 1):
                    P2 = psum.tile([128, 1024], f32, tag="q2", name=f"p2{nm}{g}")
                    pass2_half(T1, P2, g)
                    ps.append(P2)
                return ps

            # ---------------- img stage ----------------
            xtiles = {}
            for dd in range(min(2, D)):
                Xt = xin.tile([128, 2048], bf16, tag="x", name=f"x{dd}")
                ld(Xt, feat_d[dd])
                xtiles[dd] = Xt

            I2 = shared.tile([128, 2048], bf16)
            nc.vector.tensor_mul(I2[:], I[:], I[:])

            p2i = blur_p2(blur_p1(I, "i"), "i")
            mIs = shared.tile([128, 2048], bf16)
            for g in (0, 1):
                nc.scalar.activation(mIs[:, 1024 * g:1024 * (g + 1)], p2i[g][:],
                                     Act.Copy, 0.0, U)
            p2j = blur_p2(blur_p1(I2, "j"), "j")
            m2 = shared.tile([128, 2048], f32)
            nc.vector.tensor_mul(m2[:], mIs[:], mIs[:])
            varp = shared.tile([128, 2048], f32)
            for g in (0, 1):
                sl = slice(1024 * g, 1024 * (g + 1))
                nc.vector.scalar_tensor_tensor(
                    varp[:, sl], p2j[g][:], U, m2[:, sl],
                    op0=Alu.mult, op1=Alu.subtract)
            nc.vector.tensor_scalar_max(varp[:], varp[:], VAR_FLOOR)
            R = shared.tile([128, 2048], f32)
            nc.vector.reciprocal_approx_fast(R[:], varp[:])
            RS = shared.tile([128, 2048], bf16)
            nc.vector.tensor_scalar_mul(RS[:], R[:], U)
            mIR = shared.tile([128, 2048], bf16)
            nc.vector.tensor_mul(mIR[:], mIs[:], R[:])

            pd_t = {}
            Pd0 = chan.tile([128, 2048], bf16, tag="pd", name="pd0")
            nc.gpsimd.tensor_mul(Pd0[:], xtiles[0][:], I[:])
            pd_t[0] = Pd0

            mp_t, t2_t, ab_t, p2ab_t = {}, {}, {}, {}

            def phase1(d):
                if d + 1 < D:
                    Xn = xin.tile([128, 2048], bf16, tag="x", name=f"x{d+1}")
                    ld(Xn, feat_d[d + 1])
                    xtiles[d + 1] = Xn
                X = xtiles[d]
                Pd = pd_t[d]
                T1x = blur_p1(X, f"x{d}")
                p2x = blur_p2(T1x, f"x{d}")
                mp = chan.tile([128, 2048], bf16, tag="mp", name=f"mp{d}")
                for g in (0, 1):
                    nc.scalar.activation(mp[:, 1024 * g:1024 * (g + 1)],
                                         p2x[g][:], Act.Copy, 0.0, U)  # E_mp
                mp_t[d] = mp
                T1q = blur_p1(Pd, f"q{d}")
                p2p = blur_p2(T1q, f"q{d}")
                return p2p

            def evac_vo(d):
                P2a, P2b = p2ab_t[d]
                v = chan.tile([128, 2048], bf16, tag="v", name=f"v{d}")
                o = chan.tile([128, 2048], bf16, tag="o", name=f"o{d}")
                for g in (0, 1):
                    sl = slice(1024 * g, 1024 * (g + 1))
                    nc.vector.scalar_tensor_tensor(
                        v[:, sl], P2a[g][:], U, I[:, sl],
                        op0=Alu.mult, op1=Alu.mult)              # E_v (DVE)
                for g in (0, 1):
                    sl = slice(1024 * g, 1024 * (g + 1))
                    nc.vector.scalar_tensor_tensor(
                        o[:, sl], P2b[g][:], U, v[:, sl],
                        op0=Alu.mult, op1=Alu.add)               # E_o (DVE)
                st(out_d[d], o)

            def chain_b(d):
                mp = mp_t[d]
                a, u2 = ab_t[d]
                b = chan.tile([128, 2048], bf16, tag="b", name=f"b{d}")
                nc.vector.tensor_sub(b[:], mp[:], u2[:])
                ab_t[d] = (a, b)

            def chain_head(d, p2p):
                mp = mp_t[d]
                t1m = chan.tile([128, 2048], bf16, tag="t1m", name=f"t1m{d}")
                nc.vector.tensor_mul(t1m[:], mp[:], mIR[:])
                t2 = chan.tile([128, 2048], bf16, tag="t2", name=f"t2{d}")
                for g in (0, 1):
                    sl = slice(1024 * g, 1024 * (g + 1))
                    nc.vector.tensor_mul(t2[:, sl], p2p[g][:], RS[:, sl])  # E_t2
                a = chan.tile([128, 2048], bf16, tag="a", name=f"a{d}")
                nc.vector.tensor_sub(a[:], t2[:], t1m[:])
                u2 = chan.tile([128, 2048], bf16, tag="u2", name=f"u2{d}")
                nc.gpsimd.tensor_mul(u2[:], a[:], mIs[:])   # Pool
                ab_t[d] = (a, u2)

            def phase2_blur(d):
                a, b = ab_t[d]
                p2a = blur_p2(blur_p1(a, f"a{d}"), f"a{d}")
                p2b = blur_p2(blur_p1(b, f"b{d}"), f"b{d}")
                p2ab_t[d] = (p2a, p2b)

            def prefetch_pd(d):
                if d < D:
                    Pd = chan.tile([128, 2048], bf16, tag="pd", name=f"pd{d}")
                    nc.gpsimd.tensor_mul(Pd[:], xtiles[d][:], I[:])  # Pool
                    pd_t[d] = Pd

            # Steady-state slot d: phase1(d) | E_v/E_o(d-3) | b(d-1) |
            # chain head (t1m, t2, a)(d) + u2(d) on Pool | blurs of (d-2).
            for d in range(D):
                p2p = phase1(d)
                if d >= 3:
                    evac_vo(d - 3)
                if d >= 1:
                    chain_b(d - 1)
                chain_head(d, p2p)
                if d >= 2:
                    phase2_blur(d - 2)
                prefetch_pd(d + 1)
            chain_b(D - 1)
            phase2_blur(D - 2)
            evac_vo(D - 3)
            phase2_blur(D - 1)
            evac_vo(D - 2)
            evac_vo(D - 1)

    nc.compile()
    return nc


_NC_CACHE = None


def kernel(feat: np.ndarray, img: np.ndarray) -> np.ndarray:
    global _NC_CACHE
    from concourse.bass_utils import run_bass_kernel_spmd

    if _NC_CACHE is None:
        _NC_CACHE = _build_bass()
    nc = _NC_CACHE
    g = _g_packed()
    bf = ml_dtypes.bfloat16
    featb = np.ascontiguousarray(np.asarray(feat, np.float32)).astype(bf)
    imgb = np.ascontiguousarray(np.asarray(img, np.float32)).astype(bf)
    in_maps = [
        {"feat": featb[c], "img": imgb[c, 0], "gmat": g} for c in range(NCORES)
    ]
    res = run_bass_kernel_spmd(nc, in_maps, list(range(NCORES)))
    return np.stack(
        [res.results[c]["out"].astype(np.float32) for c in range(NCORES)], axis=0)
